# revision 19
# baseline (speedup 1.0000x reference)
"""Cost-volume kernel for Trainium2 (8 NeuronCores, batch-parallel).

out[b, k, h, w] = (1/(C*81)) * sum_c x[b,c,h,w] * warped[b,c,h+di,w+dj]
for the 81 offsets (di,dj) in [-4,4]^2 (zero-padded), B=8 -> one batch
element per core.

Device-side algorithm (per core), "col-tiled" final:
  - the image is tiled into 4x8 x-tiles (M=32). One PSUM block [128,192]
    holds 4 adjacent tiles (same tile-row, 4 consecutive tile-cols) via
    4 PE *column-tiled* matmuls (tile_position=(0,32j)) that run
    concurrently on independent 128x32 sub-arrays. Each matmul:
    lhsT = x-tile [C=128, 32] (tile-contiguous x layout, 1D AP — the
    stationary operand must be single-free-dim), rhs = the tile's
    12x16 window of the zero-padded warped image as a 2D-strided AP
    [C, 12, 16] (supported for the moving operand only).
  - this shrinks the per-position dump from 384 (8x16 tiles) to 192
    values -> the DRAM dump halves to 12.6 MB/core (hard floor: walrus
    rejects PSUM partition offsets that are not 32-aligned, so M=32 is
    the smallest packable tile and N=192 the smallest window).
  - warped is staged as 4 NON-overlapping row-bands of the padded
    image ([0,36/68/100,136)); windows straddling a band edge become
    TWO partial-window matmuls writing disjoint PSUM column ranges
    (zero halo re-read, no extra PE cycles, +192 instructions).
  - PSUM pool tiles of [128, 2048] (4 banks, bufs=2) hold 8 blocks
    (2 per 512-f32 bank at 0/192); one strided ACT/DVE scaled copy
    (alternating) drains 8 blocks -> SBUF bf16 [128, 1536] -> one DMA
    store (32 stores of 393KB round-robin on sync/scalar/gpsimd).
  - final relayout [81, H, W] is a constant-stride view on the host.

Measured: 102.5-102.8 us (baseline 147.5): DMA-bound at the HBM
roofline — 30.2 MB/core (x 8.4 + warped 9.2 + dump 12.6) at ~353 GB/s
= 86 us busy, plus ~8 us fixed NEFF preamble and ~5 us pipeline tail.
"""

import numpy as np

B = 8
C, H, W = 128, 128, 256
R = 4
K = 2 * R + 1  # 9
NOFF = K * K  # 81
TH, TW = 4, 8  # x-tile shape (M = 32)
NH, NW = TH + 2 * R, TW + 2 * R  # window 12 x 16
N = NH * NW  # 192
SCALE = 1.0 / (C * NOFF)

NT_H, NT_W = H // TH, W // TW  # 32 x 32 tile grid
TPB = 4  # tiles per PSUM block (4 col-tiles)
BPG = 8  # blocks per drain group (4 PSUM banks)
PW = W + 2 * R  # 264 padded cols
# Non-overlapping warped row-bands (no halo re-read). Windows that
# straddle a band edge are computed by TWO partial-window matmuls
# writing disjoint PSUM column ranges.
W_EDGES = [0, 36, 68, 100, 136]
X_TRB = 8  # tile-rows per x band
NGROUPS = NT_H * (NT_W // TPB) // BPG  # 32

PRECISION = "bf16"

_CACHE = {}


def _build_module(n_cores):
    import concourse.bacc as bacc
    import concourse.mybir as mybir
    import concourse.tile as tile

    dt = mybir.dt.bfloat16
    f32 = mybir.dt.float32
    # f32 offsets of the 8 blocks inside a [128, 2048] (4-bank) psum
    # tile: 2 blocks per 512-f32 bank at bank-internal 0/192.
    BLK_OFF = [(s // 2) * 512 + (s % 2) * 192 for s in range(8)]

    nc = bacc.Bacc(
        "TRN2", target_bir_lowering=False, debug=False, num_devices=n_cores
    )
    # x: tile-contiguous [C, nt_h, nt_w, TH*TW]; warped: padded row-major
    # [C, 136, 264]. Both host-prepped so every DMA is fully contiguous.
    x_d = nc.dram_tensor(
        "x", [C, NT_H * NT_W * TH * TW], dt, kind="ExternalInput"
    ).ap()
    w_d = nc.dram_tensor(
        "warped", [C, (H + 2 * R) * PW], dt, kind="ExternalInput"
    ).ap()
    out_d = nc.dram_tensor(
        "dump", [128, NGROUPS * BPG * N], dt, kind="ExternalOutput"
    ).ap()

    x_band_elems = X_TRB * NT_W * TH * TW  # 8192

    with tile.TileContext(nc) as tc:
        with (
            tc.tile_pool(name="wband", bufs=3) as wb_pool,
            tc.tile_pool(name="xband", bufs=3) as xb_pool,
            tc.tile_pool(name="dump", bufs=8) as dump_pool,
            tc.tile_pool(name="psum", bufs=2, space="PSUM") as psum_pool,
        ):
            store_engines = [nc.scalar, nc.gpsimd]
            # warped band tiles, loaded lazily in tile-row order
            wtiles = [None] * (len(W_EDGES) - 1)
            w2s = [None] * (len(W_EDGES) - 1)

            def get_wband(b):
                if wtiles[b] is None:
                    lo, hi = W_EDGES[b], W_EDGES[b + 1]
                    t = wb_pool.tile([128, (hi - lo) * PW], dt)
                    # skip reading the all-zero padding rows (top 4 /
                    # bottom 4 of the padded image): memset them in SBUF
                    # and DMA only the data rows.
                    zlo = max(lo, R)
                    zhi = min(hi, H + R)
                    if zlo > lo:
                        nc.vector.memset(t[:, 0 : (zlo - lo) * PW], 0.0)
                    if zhi < hi:
                        nc.vector.memset(t[:, (zhi - lo) * PW :], 0.0)
                    nc.sync.dma_start(
                        out=t[:, (zlo - lo) * PW : (zhi - lo) * PW],
                        in_=w_d[:, zlo * PW : zhi * PW],
                    )
                    wtiles[b] = t
                    w2s[b] = t[:].rearrange("p (h w) -> p h w", w=PW)
                return w2s[b]

            xtiles = [None] * (NT_H // X_TRB)

            def get_xband(b):
                if xtiles[b] is None:
                    t = xb_pool.tile([128, x_band_elems], dt)
                    # band 0 on scalar (parallel with w0 on sync at startup);
                    # all later loads ride sync so stores never queue behind
                    # a prefetched load on the store rings (ring FIFO HOL).
                    eng = nc.scalar if b == 0 else nc.sync
                    eng.dma_start(
                        out=t,
                        in_=x_d[:, b * x_band_elems : (b + 1) * x_band_elems],
                    )
                    xtiles[b] = t
                return xtiles[b]

            gidx = 0
            blk = 0
            ps = None
            for ltr in range(NT_H):  # global tile-row
                r0 = ltr * TH  # first padded window row
                # band(s) covering rows [r0, r0+NH)
                b0 = max(i for i in range(len(W_EDGES) - 1) if W_EDGES[i] <= r0)
                split = r0 + NH > W_EDGES[b0 + 1]
                xsb = get_xband(ltr // X_TRB)
                # prefetch bands needed by the NEXT tile-row (one row lead)
                if ltr + 1 < NT_H:
                    rn = (ltr + 1) * TH
                    bn = max(
                        i for i in range(len(W_EDGES) - 1) if W_EDGES[i] <= rn
                    )
                    get_wband(bn)
                    if rn + NH > W_EDGES[bn + 1]:
                        get_wband(bn + 1)
                    get_xband((ltr + 1) // X_TRB)
                for q in range(NT_W // TPB):
                    s = blk % BPG
                    if s == 0:
                        ps = psum_pool.tile([128, 2048], f32)
                    for j in range(TPB):
                        itw = q * TPB + j
                        xoff = ((ltr % X_TRB) * NT_W + itw) * (TH * TW)
                        lhsT = xsb[:, xoff : xoff + TH * TW]
                        pj = ps[32 * j : 32 * (j + 1), :]
                        if not split:
                            w2 = get_wband(b0)
                            rhs = w2[
                                :,
                                r0 - W_EDGES[b0] : r0 - W_EDGES[b0] + NH,
                                itw * TW : itw * TW + NW,
                            ]
                            nc.tensor.matmul(
                                pj[:, BLK_OFF[s] : BLK_OFF[s] + N],
                                lhsT, rhs, start=True, stop=True,
                                tile_position=(0, 32 * j),
                            )
                        else:
                            edge = W_EDGES[b0 + 1]
                            ra = edge - r0  # rows from band b0
                            w2a = get_wband(b0)
                            w2b = get_wband(b0 + 1)
                            rhs_a = w2a[
                                :,
                                r0 - W_EDGES[b0] : edge - W_EDGES[b0],
                                itw * TW : itw * TW + NW,
                            ]
                            rhs_b = w2b[
                                :, 0 : r0 + NH - edge,
                                itw * TW : itw * TW + NW,
                            ]
                            nc.tensor.matmul(
                                pj[:, BLK_OFF[s] : BLK_OFF[s] + ra * NW],
                                lhsT, rhs_a, start=True, stop=True,
                                tile_position=(0, 32 * j),
                            )
                            nc.tensor.matmul(
                                pj[:, BLK_OFF[s] + ra * NW : BLK_OFF[s] + N],
                                lhsT, rhs_b, start=True, stop=True,
                                tile_position=(0, 32 * j),
                            )
                    blk += 1
                    if s == BPG - 1:
                        db = dump_pool.tile([128, BPG * N], dt)
                        src4 = ps[:].rearrange(
                            "p (b x) -> p b x", b=4
                        )[:, :, 0 : 2 * N]
                        dst4 = db[:].rearrange("p (b x) -> p b x", b=4)
                        if gidx % 2 == 0:
                            nc.scalar.mul(dst4, src4, SCALE)
                        else:
                            nc.vector.tensor_scalar_mul(dst4, src4, SCALE)
                        eng = store_engines[gidx % len(store_engines)]
                        eng.dma_start(
                            out=out_d[
                                :, gidx * BPG * N : (gidx + 1) * BPG * N
                            ],
                            in_=db,
                        )
                        gidx += 1
            assert gidx == NGROUPS

    nc.compile()
    return nc


def _host_prep(x_b, warped_b):
    """x: [c,h,w] -> tile-contiguous [c, nt_h*nt_w*32]; warped -> padded
    row-major [c, 136*264]."""
    c = x_b.shape[0]
    xt = np.ascontiguousarray(
        x_b.reshape(c, NT_H, TH, NT_W, TW).transpose(0, 1, 3, 2, 4)
    ).reshape(c, NT_H * NT_W * TH * TW)
    wp = np.zeros((c, H + 2 * R, PW), dtype=x_b.dtype)
    wp[:, R : R + H, R : R + W] = warped_b
    return xt, wp.reshape(c, (H + 2 * R) * PW)


def _extract(dump):
    """[128, NGROUPS*BPG*N] -> [81, H, W] constant-stride view.

    dump element [m, g, s*N + n]:
      m = 32*j + hx*TW + wx ; n = (hx+di)*NW + (wx+dj)
      g = ltr ; s = q
      h = ltr*4 + hx ;  w = (q*4+j)*8 + wx
    """
    dmp = np.ascontiguousarray(dump).reshape(128, NGROUPS, BPG * N)
    sm, sg, sn = dmp.strides
    sn_e = sn  # innermost element stride (bytes)
    view = np.lib.stride_tricks.as_strided(
        dmp,
        shape=(K, K, NT_H, TH, 8, TPB, TW),
        #      di dj ltr   hx  q  j   wx
        strides=(
            NW * sn_e,            # di
            sn_e,                 # dj
            sg,                   # ltr
            TW * sm + NW * sn_e,  # hx
            N * sn_e,             # q (= s)
            32 * sm,              # j
            sm + sn_e,            # wx
        ),
    )
    # [di,dj, ltr,hx, q,j,wx] -> [81, H, W]
    out = np.ascontiguousarray(view).reshape(NOFF, H, W)
    return out.astype(np.float32)


def kernel(x, warped):
    from concourse import bass_utils

    x = np.asarray(x, dtype=np.float32)
    warped = np.asarray(warped, dtype=np.float32)
    assert x.shape == (B, C, H, W) and warped.shape == (B, C, H, W)

    import ml_dtypes

    x = x.astype(ml_dtypes.bfloat16)
    warped = warped.astype(ml_dtypes.bfloat16)

    key = "v3"
    if key not in _CACHE:
        _CACHE[key] = _build_module(B)
    nc = _CACHE[key]

    in_maps = []
    for b in range(B):
        xt, wp = _host_prep(x[b], warped[b])
        in_maps.append({"x": xt, "warped": wp})
    res = bass_utils.run_bass_kernel_spmd(nc, in_maps, core_ids=list(range(B)))
    global LAST_RESULTS
    LAST_RESULTS = res
    out = np.empty((B, NOFF, H, W), dtype=np.float32)
    for b in range(B):
        out[b] = _extract(res.results[b]["dump"])
    return out


# revision 20
# speedup vs baseline: 1.0740x; 1.0740x over previous
"""Cost-volume kernel for Trainium2 (8 NeuronCores, batch-parallel).

out[b, k, h, w] = (1/(C*81)) * sum_c x[b,c,h,w] * warped[b,c,h+di,w+dj]
for the 81 offsets (di,dj) in [-4,4]^2 (zero-padded), B=8 -> one batch
element per core.

Device-side algorithm (per core), "col-tiled" final:
  - the image is tiled into 4x8 x-tiles (M=32). One PSUM block [128,192]
    holds 4 adjacent tiles (same tile-row, 4 consecutive tile-cols) via
    4 PE *column-tiled* matmuls (tile_position=(0,32j)) that run
    concurrently on independent 128x32 sub-arrays. Each matmul:
    lhsT = x-tile [C=128, 32] (tile-contiguous x layout, 1D AP — the
    stationary operand must be single-free-dim), rhs = the tile's
    12x16 window of the zero-padded warped image as a 2D-strided AP
    [C, 12, 16] (supported for the moving operand only).
  - this shrinks the per-position dump from 384 (8x16 tiles) to 192
    values -> the DRAM dump halves to 12.6 MB/core (hard floor: walrus
    rejects PSUM partition offsets that are not 32-aligned, so M=32 is
    the smallest packable tile and N=192 the smallest window).
  - warped is staged as 4 NON-overlapping row-bands of the padded
    image ([0,36/68/100,136)); windows straddling a band edge become
    TWO partial-window matmuls writing disjoint PSUM column ranges
    (zero halo re-read, no extra PE cycles, +192 instructions).
  - PSUM pool tiles of [128, 2048] (4 banks, bufs=2) hold 8 blocks
    (2 per 512-f32 bank at 0/192); one strided ACT/DVE scaled copy
    (alternating) drains 8 blocks -> SBUF bf16 [128, 1536] -> one DMA
    store (32 stores of 393KB round-robin on sync/scalar/gpsimd).
  - final relayout [81, H, W] is a constant-stride view on the host.

Measured: 102.5-102.8 us (baseline 147.5): DMA-bound at the HBM
roofline — 30.2 MB/core (x 8.4 + warped 9.2 + dump 12.6) at ~353 GB/s
= 86 us busy, plus ~8 us fixed NEFF preamble and ~5 us pipeline tail.
"""

import numpy as np

B = 8
C, H, W = 128, 128, 256
R = 4
K = 2 * R + 1  # 9
NOFF = K * K  # 81
TH, TW = 4, 8  # x-tile shape (M = 32)
NH, NW = TH + 2 * R, TW + 2 * R  # window 12 x 16
N = NH * NW  # 192
SCALE = 1.0 / (C * NOFF)

NT_H, NT_W = H // TH, W // TW  # 32 x 32 tile grid
TPB = 4  # tiles per PSUM block (4 col-tiles)
BPG = 8  # blocks per drain group (4 PSUM banks)
PW = W + 2 * R  # 264 padded cols
# Non-overlapping warped row-bands (no halo re-read). Windows that
# straddle a band edge are computed by TWO partial-window matmuls
# writing disjoint PSUM column ranges.
W_EDGES = [0, 36, 68, 100, 136]
X_TRB = 8  # tile-rows per x band
NGROUPS = NT_H * (NT_W // TPB) // BPG  # 32

PRECISION = "bf16"

_CACHE = {}


def _build_module(n_cores):
    import concourse.bacc as bacc
    import concourse.mybir as mybir
    import concourse.tile as tile

    dt = mybir.dt.bfloat16
    f32 = mybir.dt.float32
    # f32 offsets of the 8 blocks inside a [128, 2048] (4-bank) psum
    # tile: 2 blocks per 512-f32 bank at bank-internal 0/192.
    BLK_OFF = [(s // 2) * 512 + (s % 2) * 192 for s in range(8)]

    nc = bacc.Bacc(
        "TRN2", target_bir_lowering=False, debug=False, num_devices=n_cores
    )
    # x: tile-contiguous [C, nt_h, nt_w, TH*TW]; warped: padded row-major
    # [C, 136, 264]. Both host-prepped so every DMA is fully contiguous.
    x_d = nc.dram_tensor(
        "x", [C, NT_H * NT_W * TH * TW], dt, kind="ExternalInput"
    ).ap()
    w_d = nc.dram_tensor(
        "warped", [C, (H + 2 * R) * PW], dt, kind="ExternalInput"
    ).ap()
    out_d = nc.dram_tensor(
        "dump", [128, NGROUPS * BPG * N], dt, kind="ExternalOutput"
    ).ap()

    x_band_elems = X_TRB * NT_W * TH * TW  # 8192

    with tile.TileContext(nc) as tc:
        with (
            tc.tile_pool(name="wband", bufs=3) as wb_pool,
            tc.tile_pool(name="xband", bufs=3) as xb_pool,
            tc.tile_pool(name="dump", bufs=8) as dump_pool,
            tc.tile_pool(name="psum", bufs=2, space="PSUM") as psum_pool,
        ):
            store_engines = [nc.scalar, nc.gpsimd]
            # warped band tiles, loaded lazily in tile-row order
            wtiles = [None] * (len(W_EDGES) - 1)
            w2s = [None] * (len(W_EDGES) - 1)

            def get_wband(b):
                if wtiles[b] is None:
                    lo, hi = W_EDGES[b], W_EDGES[b + 1]
                    t = wb_pool.tile([128, (hi - lo) * PW], dt)
                    nc.sync.dma_start(out=t, in_=w_d[:, lo * PW : hi * PW])
                    wtiles[b] = t
                    w2s[b] = t[:].rearrange("p (h w) -> p h w", w=PW)
                return w2s[b]

            xtiles = [None] * (NT_H // X_TRB)

            def get_xband(b):
                if xtiles[b] is None:
                    t = xb_pool.tile([128, x_band_elems], dt)
                    # band 0 on scalar (parallel with w0 on sync at startup);
                    # all later loads ride sync so stores never queue behind
                    # a prefetched load on the store rings (ring FIFO HOL).
                    eng = nc.scalar if b == 0 else nc.sync
                    eng.dma_start(
                        out=t,
                        in_=x_d[:, b * x_band_elems : (b + 1) * x_band_elems],
                    )
                    xtiles[b] = t
                return xtiles[b]

            gidx = 0
            blk = 0
            ps = None
            for ltr in range(NT_H):  # global tile-row
                r0 = ltr * TH  # first padded window row
                # band(s) covering rows [r0, r0+NH)
                b0 = max(i for i in range(len(W_EDGES) - 1) if W_EDGES[i] <= r0)
                split = r0 + NH > W_EDGES[b0 + 1]
                xsb = get_xband(ltr // X_TRB)
                # prefetch bands needed by the NEXT tile-row (one row lead)
                if ltr + 1 < NT_H:
                    rn = (ltr + 1) * TH
                    bn = max(
                        i for i in range(len(W_EDGES) - 1) if W_EDGES[i] <= rn
                    )
                    get_wband(bn)
                    if rn + NH > W_EDGES[bn + 1]:
                        get_wband(bn + 1)
                    get_xband((ltr + 1) // X_TRB)
                for q in range(NT_W // TPB):
                    s = blk % BPG
                    if s == 0:
                        ps = psum_pool.tile([128, 2048], f32)
                    for j in range(TPB):
                        itw = q * TPB + j
                        xoff = ((ltr % X_TRB) * NT_W + itw) * (TH * TW)
                        lhsT = xsb[:, xoff : xoff + TH * TW]
                        pj = ps[32 * j : 32 * (j + 1), :]
                        if not split:
                            w2 = get_wband(b0)
                            rhs = w2[
                                :,
                                r0 - W_EDGES[b0] : r0 - W_EDGES[b0] + NH,
                                itw * TW : itw * TW + NW,
                            ]
                            nc.tensor.matmul(
                                pj[:, BLK_OFF[s] : BLK_OFF[s] + N],
                                lhsT, rhs, start=True, stop=True,
                                tile_position=(0, 32 * j),
                            )
                        else:
                            edge = W_EDGES[b0 + 1]
                            ra = edge - r0  # rows from band b0
                            w2a = get_wband(b0)
                            w2b = get_wband(b0 + 1)
                            rhs_a = w2a[
                                :,
                                r0 - W_EDGES[b0] : edge - W_EDGES[b0],
                                itw * TW : itw * TW + NW,
                            ]
                            rhs_b = w2b[
                                :, 0 : r0 + NH - edge,
                                itw * TW : itw * TW + NW,
                            ]
                            nc.tensor.matmul(
                                pj[:, BLK_OFF[s] : BLK_OFF[s] + ra * NW],
                                lhsT, rhs_a, start=True, stop=True,
                                tile_position=(0, 32 * j),
                            )
                            nc.tensor.matmul(
                                pj[:, BLK_OFF[s] + ra * NW : BLK_OFF[s] + N],
                                lhsT, rhs_b, start=True, stop=True,
                                tile_position=(0, 32 * j),
                            )
                    blk += 1
                    if s == BPG - 1:
                        db = dump_pool.tile([128, BPG * N], dt)
                        src4 = ps[:].rearrange(
                            "p (b x) -> p b x", b=4
                        )[:, :, 0 : 2 * N]
                        dst4 = db[:].rearrange("p (b x) -> p b x", b=4)
                        if gidx % 2 == 0:
                            nc.scalar.mul(dst4, src4, SCALE)
                        else:
                            nc.vector.tensor_scalar_mul(dst4, src4, SCALE)
                        eng = store_engines[gidx % len(store_engines)]
                        eng.dma_start(
                            out=out_d[
                                :, gidx * BPG * N : (gidx + 1) * BPG * N
                            ],
                            in_=db,
                        )
                        gidx += 1
            assert gidx == NGROUPS

    nc.compile()
    return nc


def _host_prep(x_b, warped_b):
    """x: [c,h,w] -> tile-contiguous [c, nt_h*nt_w*32]; warped -> padded
    row-major [c, 136*264]."""
    c = x_b.shape[0]
    xt = np.ascontiguousarray(
        x_b.reshape(c, NT_H, TH, NT_W, TW).transpose(0, 1, 3, 2, 4)
    ).reshape(c, NT_H * NT_W * TH * TW)
    wp = np.zeros((c, H + 2 * R, PW), dtype=x_b.dtype)
    wp[:, R : R + H, R : R + W] = warped_b
    return xt, wp.reshape(c, (H + 2 * R) * PW)


def _extract(dump):
    """[128, NGROUPS*BPG*N] -> [81, H, W] constant-stride view.

    dump element [m, g, s*N + n]:
      m = 32*j + hx*TW + wx ; n = (hx+di)*NW + (wx+dj)
      g = ltr ; s = q
      h = ltr*4 + hx ;  w = (q*4+j)*8 + wx
    """
    dmp = np.ascontiguousarray(dump).reshape(128, NGROUPS, BPG * N)
    sm, sg, sn = dmp.strides
    sn_e = sn  # innermost element stride (bytes)
    view = np.lib.stride_tricks.as_strided(
        dmp,
        shape=(K, K, NT_H, TH, 8, TPB, TW),
        #      di dj ltr   hx  q  j   wx
        strides=(
            NW * sn_e,            # di
            sn_e,                 # dj
            sg,                   # ltr
            TW * sm + NW * sn_e,  # hx
            N * sn_e,             # q (= s)
            32 * sm,              # j
            sm + sn_e,            # wx
        ),
    )
    # [di,dj, ltr,hx, q,j,wx] -> [81, H, W]
    out = np.ascontiguousarray(view).reshape(NOFF, H, W)
    return out.astype(np.float32)


def kernel(x, warped):
    from concourse import bass_utils

    x = np.asarray(x, dtype=np.float32)
    warped = np.asarray(warped, dtype=np.float32)
    assert x.shape == (B, C, H, W) and warped.shape == (B, C, H, W)

    import ml_dtypes

    x = x.astype(ml_dtypes.bfloat16)
    warped = warped.astype(ml_dtypes.bfloat16)

    key = "v3"
    if key not in _CACHE:
        _CACHE[key] = _build_module(B)
    nc = _CACHE[key]

    in_maps = []
    for b in range(B):
        xt, wp = _host_prep(x[b], warped[b])
        in_maps.append({"x": xt, "warped": wp})
    res = bass_utils.run_bass_kernel_spmd(nc, in_maps, core_ids=list(range(B)))
    global LAST_RESULTS
    LAST_RESULTS = res
    out = np.empty((B, NOFF, H, W), dtype=np.float32)
    for b in range(B):
        out[b] = _extract(res.results[b]["dump"])
    return out


# revision 21
# speedup vs baseline: 1.1034x; 1.0274x over previous
"""Cost-volume kernel for Trainium2 (8 NeuronCores, batch-parallel).

out[b, k, h, w] = (1/(C*81)) * sum_c x[b,c,h,w] * warped[b,c,h+di,w+dj]
for the 81 offsets (di,dj) in [-4,4]^2 (zero-padded), B=8 -> one batch
element per core.

Device-side algorithm (per core), "col-tiled" final:
  - the image is tiled into 4x8 x-tiles (M=32). One PSUM block [128,192]
    holds 4 adjacent tiles (same tile-row, 4 consecutive tile-cols) via
    4 PE *column-tiled* matmuls (tile_position=(0,32j)) that run
    concurrently on independent 128x32 sub-arrays. Each matmul:
    lhsT = x-tile [C=128, 32] (tile-contiguous x layout, 1D AP — the
    stationary operand must be single-free-dim), rhs = the tile's
    12x16 window of the zero-padded warped image as a 2D-strided AP
    [C, 12, 16] (supported for the moving operand only).
  - this shrinks the per-position dump from 384 (8x16 tiles) to 192
    values -> the DRAM dump halves to 12.6 MB/core (hard floor: walrus
    rejects PSUM partition offsets that are not 32-aligned, so M=32 is
    the smallest packable tile and N=192 the smallest window).
  - warped is staged as 4 NON-overlapping row-bands of the padded
    image ([0,36/68/100,136)); windows straddling a band edge become
    TWO partial-window matmuls writing disjoint PSUM column ranges
    (zero halo re-read, no extra PE cycles, +192 instructions).
  - PSUM pool tiles of [128, 2048] (4 banks, bufs=2) hold 8 blocks
    (2 per 512-f32 bank at 0/192); one strided ACT/DVE scaled copy
    (alternating) drains 8 blocks -> SBUF bf16 [128, 1536] -> one DMA
    store (32 stores of 393KB round-robin on sync/scalar/gpsimd).
  - final relayout [81, H, W] is a constant-stride view on the host.

Measured: 102.5-102.8 us (baseline 147.5): DMA-bound at the HBM
roofline — 30.2 MB/core (x 8.4 + warped 9.2 + dump 12.6) at ~353 GB/s
= 86 us busy, plus ~8 us fixed NEFF preamble and ~5 us pipeline tail.
"""

import numpy as np

B = 8
C, H, W = 128, 128, 256
R = 4
K = 2 * R + 1  # 9
NOFF = K * K  # 81
TH, TW = 4, 8  # x-tile shape (M = 32)
NH, NW = TH + 2 * R, TW + 2 * R  # window 12 x 16
N = NH * NW  # 192
SCALE = 1.0 / (C * NOFF)

NT_H, NT_W = H // TH, W // TW  # 32 x 32 tile grid
TPB = 4  # tiles per PSUM block (4 col-tiles)
BPG = 8  # blocks per drain group (4 PSUM banks)
PW = W + 2 * R  # 264 padded cols
# Non-overlapping warped row-bands (no halo re-read). Windows that
# straddle a band edge are computed by TWO partial-window matmuls
# writing disjoint PSUM column ranges.
W_EDGES = [0, 32, 68, 104, 136]
X_TRB = 8  # tile-rows per x band
NGROUPS = NT_H * (NT_W // TPB) // BPG  # 32

PRECISION = "bf16"

_CACHE = {}


def _build_module(n_cores):
    import concourse.bacc as bacc
    import concourse.mybir as mybir
    import concourse.tile as tile

    dt = mybir.dt.bfloat16
    f32 = mybir.dt.float32
    # f32 offsets of the 8 blocks inside a [128, 2048] (4-bank) psum
    # tile: 2 blocks per 512-f32 bank at bank-internal 0/192.
    BLK_OFF = [(s // 2) * 512 + (s % 2) * 192 for s in range(8)]

    nc = bacc.Bacc(
        "TRN2", target_bir_lowering=False, debug=False, num_devices=n_cores
    )
    # x: tile-contiguous [C, nt_h, nt_w, TH*TW]; warped: padded row-major
    # [C, 136, 264]. Both host-prepped so every DMA is fully contiguous.
    x_d = nc.dram_tensor(
        "x", [C, NT_H * NT_W * TH * TW], dt, kind="ExternalInput"
    ).ap()
    w_d = nc.dram_tensor(
        "warped", [C, (H + 2 * R) * PW], dt, kind="ExternalInput"
    ).ap()
    out_d = nc.dram_tensor(
        "dump", [128, NGROUPS * BPG * N], dt, kind="ExternalOutput"
    ).ap()

    x_band_elems = X_TRB * NT_W * TH * TW  # 8192

    with tile.TileContext(nc) as tc:
        with (
            tc.tile_pool(name="wband", bufs=3) as wb_pool,
            tc.tile_pool(name="xband", bufs=3) as xb_pool,
            tc.tile_pool(name="dump", bufs=12) as dump_pool,
            tc.tile_pool(name="psum", bufs=2, space="PSUM") as psum_pool,
        ):
            store_engines = [nc.scalar, nc.gpsimd]
            # warped band tiles, loaded lazily in tile-row order
            wtiles = [None] * (len(W_EDGES) - 1)
            w2s = [None] * (len(W_EDGES) - 1)

            def get_wband(b):
                if wtiles[b] is None:
                    lo, hi = W_EDGES[b], W_EDGES[b + 1]
                    t = wb_pool.tile([128, (hi - lo) * PW], dt)
                    nc.sync.dma_start(out=t, in_=w_d[:, lo * PW : hi * PW])
                    wtiles[b] = t
                    w2s[b] = t[:].rearrange("p (h w) -> p h w", w=PW)
                return w2s[b]

            xtiles = [None] * (NT_H // X_TRB)

            def get_xband(b):
                if xtiles[b] is None:
                    t = xb_pool.tile([128, x_band_elems], dt)
                    # band 0 on scalar (parallel with w0 on sync at startup);
                    # all later loads ride sync so stores never queue behind
                    # a prefetched load on the store rings (ring FIFO HOL).
                    eng = nc.scalar if b == 0 else nc.sync
                    eng.dma_start(
                        out=t,
                        in_=x_d[:, b * x_band_elems : (b + 1) * x_band_elems],
                    )
                    xtiles[b] = t
                return xtiles[b]

            gidx = 0
            blk = 0
            ps = None
            for ltr in range(NT_H):  # global tile-row
                r0 = ltr * TH  # first padded window row
                # band(s) covering rows [r0, r0+NH)
                b0 = max(i for i in range(len(W_EDGES) - 1) if W_EDGES[i] <= r0)
                split = r0 + NH > W_EDGES[b0 + 1]
                xsb = get_xband(ltr // X_TRB)
                # prefetch bands needed by the NEXT tile-row (one row lead)
                if ltr + 1 < NT_H:
                    rn = (ltr + 1) * TH
                    bn = max(
                        i for i in range(len(W_EDGES) - 1) if W_EDGES[i] <= rn
                    )
                    get_wband(bn)
                    if rn + NH > W_EDGES[bn + 1]:
                        get_wband(bn + 1)
                    get_xband((ltr + 1) // X_TRB)
                for q in range(NT_W // TPB):
                    s = blk % BPG
                    if s == 0:
                        ps = psum_pool.tile([128, 2048], f32)
                    for j in range(TPB):
                        itw = q * TPB + j
                        xoff = ((ltr % X_TRB) * NT_W + itw) * (TH * TW)
                        lhsT = xsb[:, xoff : xoff + TH * TW]
                        pj = ps[32 * j : 32 * (j + 1), :]
                        if not split:
                            w2 = get_wband(b0)
                            rhs = w2[
                                :,
                                r0 - W_EDGES[b0] : r0 - W_EDGES[b0] + NH,
                                itw * TW : itw * TW + NW,
                            ]
                            nc.tensor.matmul(
                                pj[:, BLK_OFF[s] : BLK_OFF[s] + N],
                                lhsT, rhs, start=True, stop=True,
                                tile_position=(0, 32 * j),
                            )
                        else:
                            edge = W_EDGES[b0 + 1]
                            ra = edge - r0  # rows from band b0
                            w2a = get_wband(b0)
                            w2b = get_wband(b0 + 1)
                            rhs_a = w2a[
                                :,
                                r0 - W_EDGES[b0] : edge - W_EDGES[b0],
                                itw * TW : itw * TW + NW,
                            ]
                            rhs_b = w2b[
                                :, 0 : r0 + NH - edge,
                                itw * TW : itw * TW + NW,
                            ]
                            nc.tensor.matmul(
                                pj[:, BLK_OFF[s] : BLK_OFF[s] + ra * NW],
                                lhsT, rhs_a, start=True, stop=True,
                                tile_position=(0, 32 * j),
                            )
                            nc.tensor.matmul(
                                pj[:, BLK_OFF[s] + ra * NW : BLK_OFF[s] + N],
                                lhsT, rhs_b, start=True, stop=True,
                                tile_position=(0, 32 * j),
                            )
                    blk += 1
                    if s == BPG - 1:
                        db = dump_pool.tile([128, BPG * N], dt)
                        src4 = ps[:].rearrange(
                            "p (b x) -> p b x", b=4
                        )[:, :, 0 : 2 * N]
                        dst4 = db[:].rearrange("p (b x) -> p b x", b=4)
                        if gidx % 2 == 0:
                            nc.scalar.mul(dst4, src4, SCALE)
                        else:
                            nc.vector.tensor_scalar_mul(dst4, src4, SCALE)
                        eng = store_engines[gidx % len(store_engines)]
                        eng.dma_start(
                            out=out_d[
                                :, gidx * BPG * N : (gidx + 1) * BPG * N
                            ],
                            in_=db,
                        )
                        gidx += 1
            assert gidx == NGROUPS

    nc.compile()
    return nc


def _host_prep(x_b, warped_b):
    """x: [c,h,w] -> tile-contiguous [c, nt_h*nt_w*32]; warped -> padded
    row-major [c, 136*264]."""
    c = x_b.shape[0]
    xt = np.ascontiguousarray(
        x_b.reshape(c, NT_H, TH, NT_W, TW).transpose(0, 1, 3, 2, 4)
    ).reshape(c, NT_H * NT_W * TH * TW)
    wp = np.zeros((c, H + 2 * R, PW), dtype=x_b.dtype)
    wp[:, R : R + H, R : R + W] = warped_b
    return xt, wp.reshape(c, (H + 2 * R) * PW)


def _extract(dump):
    """[128, NGROUPS*BPG*N] -> [81, H, W] constant-stride view.

    dump element [m, g, s*N + n]:
      m = 32*j + hx*TW + wx ; n = (hx+di)*NW + (wx+dj)
      g = ltr ; s = q
      h = ltr*4 + hx ;  w = (q*4+j)*8 + wx
    """
    dmp = np.ascontiguousarray(dump).reshape(128, NGROUPS, BPG * N)
    sm, sg, sn = dmp.strides
    sn_e = sn  # innermost element stride (bytes)
    view = np.lib.stride_tricks.as_strided(
        dmp,
        shape=(K, K, NT_H, TH, 8, TPB, TW),
        #      di dj ltr   hx  q  j   wx
        strides=(
            NW * sn_e,            # di
            sn_e,                 # dj
            sg,                   # ltr
            TW * sm + NW * sn_e,  # hx
            N * sn_e,             # q (= s)
            32 * sm,              # j
            sm + sn_e,            # wx
        ),
    )
    # [di,dj, ltr,hx, q,j,wx] -> [81, H, W]
    out = np.ascontiguousarray(view).reshape(NOFF, H, W)
    return out.astype(np.float32)


def kernel(x, warped):
    from concourse import bass_utils

    x = np.asarray(x, dtype=np.float32)
    warped = np.asarray(warped, dtype=np.float32)
    assert x.shape == (B, C, H, W) and warped.shape == (B, C, H, W)

    import ml_dtypes

    x = x.astype(ml_dtypes.bfloat16)
    warped = warped.astype(ml_dtypes.bfloat16)

    key = "v3"
    if key not in _CACHE:
        _CACHE[key] = _build_module(B)
    nc = _CACHE[key]

    in_maps = []
    for b in range(B):
        xt, wp = _host_prep(x[b], warped[b])
        in_maps.append({"x": xt, "warped": wp})
    res = bass_utils.run_bass_kernel_spmd(nc, in_maps, core_ids=list(range(B)))
    global LAST_RESULTS
    LAST_RESULTS = res
    out = np.empty((B, NOFF, H, W), dtype=np.float32)
    for b in range(B):
        out[b] = _extract(res.results[b]["dump"])
    return out
